# revision 6
# baseline (speedup 1.0000x reference)
"""Single-head causal attention (B=4, T=4096, n_embd=1024, head=64) on 8 trn2 cores.

One SPMD program, 8 cores, one launch.  Core c -> batch b=c//2, half h=c%2.
Causal-balanced q-block (512 rows) assignment: half0 {0,3,4,7}, half1 {1,2,5,6}.

To keep the instruction stream identical across cores, each core runs 4 fixed
attention "slots" with k-ranges {8,16,24,32} k-blocks (128 keys each).  A slot
hosts one of the core's q-blocks (which one is per-core DATA, not control flow):
  half0: slots host qb {0,3,4,7} (own nk {4,16,20,32})
  half1: slots host qb {1,2,5,6} (own nk {8,12,24,28})
The last 8 k-blocks of every slot get a mask multiply; the mask tile is selected
at runtime from a 6-pattern table (tri0..3, zero, ones) via dynamic-AP offsets
loaded from a per-core int32 vector.  This zeroes both the causal diagonal and
the slot padding (own nk < slot nk).

Math (S^T formulation, all fp32):
  S^T[tk,tq] = K_blk^T.T @ Q^T          (PE, psum [128, 2*512])
  P^T = exp(S^T / 8)                    (one ACT op over both banks; no max-sub
                                         needed: S ~ N(0,1), exp can't overflow)
  P^T *= mask (last 8 kbs of slot)      (DVE, dynamic-AP pattern select)
  O_aug^T[65,512] += V_aug_blk.T @ P^T  (PE; V_aug col 64 = ones => row 64 of
                                         O_aug accumulates the softmax denom)
Epilogue per slot: PE-transpose O_aug^T -> [128tq, 65], reciprocal of col 64,
ACT copy*scale -> natural [128,64] rows, DMA out.  Host reassembles slots.
"""

import numpy as np

B, T, NE, HD = 4, 4096, 1024, 64
QB = 512            # q-block width
KB = 128            # k-block width
NQB = T // QB       # 8 t-blocks
NT = NE // 128      # 8 n-tiles (projection contraction)
SLOT_NK = [8, 16, 24, 32]          # k-blocks per slot (pairs: 4, 8, 12, 16)
HALF_QBS = [[0, 3, 4, 7], [1, 2, 5, 6]]   # slot si hosts q-block HALF_QBS[h][si]
DUMP = 4 * QB        # dump column in qt_sel for unused panels (not used now)

_CACHE = {}


def _host_tables(half):
    """Per-core mask thresholds [32] and q-select offsets [4].

    Mask for slot si, masked-index j (k-block kx = SLOT_NK[si]-8+j):
    valid(i, c) iff qoff + c >= kx*128 + i  iff  (c - i) >= 128*kx - qoff.
    """
    thr = np.zeros(32, dtype=np.float32)
    qoffs = np.zeros(4, dtype=np.int32)
    for si, nk in enumerate(SLOT_NK):
        own_qb = HALF_QBS[half][si]
        qoffs[si] = own_qb * QB
        for j in range(8):
            kx = nk - 8 + j
            thr[si * 8 + j] = 128.0 * kx - float(qoffs[si])
    return thr, qoffs


def _build_program():
    import concourse.bass as bass
    import concourse.mybir as mybir
    import concourse.tile as tile

    f32 = mybir.dt.float32
    bf16 = mybir.dt.bfloat16
    i32 = mybir.dt.int32
    AF = mybir.ActivationFunctionType
    MS = bass.MemorySpace
    nc = bass.Bass("TRN2", target_bir_lowering=True, debug=False,
                   enable_asserts=False)

    xt_d = nc.dram_tensor("xt", [NE, T], bf16, kind="ExternalInput").ap()
    wkv_d = nc.dram_tensor("wkv", [NE, 128], bf16, kind="ExternalInput").ap()
    wq_d = nc.dram_tensor("wq", [NE, HD], bf16, kind="ExternalInput").ap()
    ident_d = nc.dram_tensor("ident", [128, 128], f32, kind="ExternalInput").ap()
    identh_d = nc.dram_tensor("identh", [128, 64], bf16, kind="ExternalInput").ap()
    dtab_d = nc.dram_tensor("dtab", [128, QB], f32, kind="ExternalInput").ap()
    thr_d = nc.dram_tensor("thr", [128, 32], f32, kind="ExternalInput").ap()
    qoffs_d = nc.dram_tensor("qoffs", [1, 4], i32, kind="ExternalInput").ap()
    out_d = nc.dram_tensor("out", [4 * QB, HD], f32, kind="ExternalOutput").ap()

    with tile.TileContext(nc) as tc:
        with (
            tc.tile_pool(name="consts", bufs=1) as cpool,
            tc.tile_pool(name="big", bufs=1) as bigpool,
            tc.tile_pool(name="xt", bufs=2) as xtpool,
            tc.tile_pool(name="pt", bufs=3) as ptpool,
            tc.tile_pool(name="osb", bufs=2) as osbpool,
            tc.tile_pool(name="onat", bufs=2) as onatpool,
            tc.tile_pool(name="rec", bufs=2) as recpool,
            tc.tile_pool(name="sps", bufs=2, space=MS.PSUM) as spool,
            tc.tile_pool(name="ops", bufs=2, space=MS.PSUM) as opool,
            tc.tile_pool(name="projps", bufs=2, space=MS.PSUM) as projpool,
        ):
            # ---- constants ----
            wkv_sb = cpool.tile([128, NT, 128], bf16)
            nc.gpsimd.dma_start(wkv_sb[:], wkv_d.rearrange("(nt p) m -> p nt m", p=128))
            wq_sb = cpool.tile([128, NT, HD], bf16)
            nc.gpsimd.dma_start(wq_sb[:], wq_d.rearrange("(nt p) m -> p nt m", p=128))
            ident = cpool.tile([128, 128], f32)
            nc.gpsimd.dma_start(ident[:], ident_d[:])
            identh = cpool.tile([128, 64], bf16)
            nc.gpsimd.dma_start(identh[:], identh_d[:])
            dtab = cpool.tile([128, QB], f32)
            nc.gpsimd.dma_start(dtab[:], dtab_d[:])
            thr = cpool.tile([128, 32], f32)
            nc.gpsimd.dma_start(thr[:], thr_d[:])
            qoffs = cpool.tile([1, 4], i32)
            nc.gpsimd.dma_start(qoffs[:], qoffs_d[:])

            # ---- persistent sbuf state ----
            kvt = bigpool.tile([128, T], bf16)          # 0:64 K^T, 64:128 V^T
            qt_all = bigpool.tile([64, T], bf16)        # Q^T all 8 panels
            qt_sel = bigpool.tile([64, 4 * QB], bf16)   # slot-ordered Q^T
            v_aug = bigpool.tile([128, 32 * 65], bf16)  # V natural + ones col
            nc.vector.memset(v_aug[:], 1.0)

            def dyn_load(ap, lo, hi):
                tmp = nc.vector.alloc_register(f"dyn{nc.next_id()}")
                nc.vector.reg_load(tmp, ap)
                return nc.vector.snap(tmp, donate=True, min_val=lo, max_val=hi)

            def emit_attention(si):
                nk = SLOT_NK[si]
                npair = nk // 2
                o_ps = opool.tile([65, QB], f32, tag="ops")
                for p in range(npair):
                    ka, kb2 = 2 * p, 2 * p + 1
                    s_ps = spool.tile([128, 2 * QB], f32, tag="sps")
                    nc.tensor.matmul(
                        s_ps[:, 0:QB],
                        kvt[0:64, ka * KB:(ka + 1) * KB],
                        qt_sel[:, si * QB:(si + 1) * QB],
                        start=True, stop=True)
                    nc.tensor.matmul(
                        s_ps[:, QB:2 * QB],
                        kvt[0:64, kb2 * KB:(kb2 + 1) * KB],
                        qt_sel[:, si * QB:(si + 1) * QB],
                        start=True, stop=True)
                    pt = ptpool.tile([128, 2 * QB], bf16, tag="pt")
                    nc.scalar.activation(pt[:], s_ps[:], AF.Exp,
                                         scale=float(HD) ** -0.5)
                    for half_i, kx in enumerate((ka, kb2)):
                        j = kx - (nk - 8)
                        if j >= 0:
                            # pt = (dtab >= thr) * pt ; thr = 128*kx - qoff
                            nc.vector.scalar_tensor_tensor(
                                pt[:, half_i * QB:(half_i + 1) * QB],
                                dtab[:],
                                thr[:, si * 8 + j: si * 8 + j + 1],
                                pt[:, half_i * QB:(half_i + 1) * QB],
                                mybir.AluOpType.is_ge,
                                mybir.AluOpType.mult)
                    nc.tensor.matmul(
                        o_ps[:], v_aug[:, ka * 65:ka * 65 + 65], pt[:, 0:QB],
                        start=(p == 0), stop=False, skip_group_check=True)
                    nc.tensor.matmul(
                        o_ps[:], v_aug[:, kb2 * 65:kb2 * 65 + 65],
                        pt[:, QB:2 * QB],
                        start=False, stop=(p == npair - 1),
                        skip_group_check=True)
                # epilogue
                ot_sb = osbpool.tile([65, QB], f32, tag="osb")
                nc.scalar.copy(ot_sb[:], o_ps[:])
                for u in range(QB // 128):
                    tp_ps = projpool.tile([128, QB], f32, tag="proj")
                    nc.tensor.transpose(
                        tp_ps[:, 0:65], ot_sb[:, u * 128:(u + 1) * 128],
                        ident[0:65, 0:65])
                    rec = recpool.tile([128, 1], f32, tag="rec")
                    nc.vector.reciprocal(rec[:], tp_ps[:, 64:65])
                    o_nat = onatpool.tile([128, HD], f32, tag="onat")
                    nc.scalar.activation(o_nat[:], tp_ps[:, 0:HD], AF.Copy,
                                         scale=rec[:])
                    nc.sync.dma_start(
                        out_d[si * QB + u * 128: si * QB + (u + 1) * 128, :],
                        o_nat[:])

            # ---- main pipeline over t-blocks ----
            for tb in range(NQB):
                xt_sb = xtpool.tile([128, NT, QB], bf16, tag="xt")
                nc.gpsimd.dma_start(
                    xt_sb[:],
                    xt_d[:, tb * QB:(tb + 1) * QB].rearrange(
                        "(nt p) t -> p nt t", p=128))
                kv_ps = projpool.tile([128, QB], f32, tag="proj")
                for ni in range(NT):
                    nc.tensor.matmul(kv_ps[:], wkv_sb[:, ni, :], xt_sb[:, ni, :],
                                     start=(ni == 0), stop=(ni == NT - 1))
                nc.vector.tensor_copy(kvt[:, tb * QB:(tb + 1) * QB], kv_ps[:])
                q_ps = projpool.tile([64, QB], f32, tag="proj")
                for ni in range(NT):
                    nc.tensor.matmul(q_ps[:], wq_sb[:, ni, :], xt_sb[:, ni, :],
                                     start=(ni == 0), stop=(ni == NT - 1))
                nc.vector.tensor_copy(qt_all[:, tb * QB:(tb + 1) * QB], q_ps[:])
                for j in range(QB // KB):
                    kb = tb * (QB // KB) + j
                    tp_ps = projpool.tile([128, 2 * QB], bf16, tag="proj")
                    nc.tensor.transpose(
                        tp_ps[:, 0:64], kvt[64:128, kb * KB:(kb + 1) * KB],
                        identh[64:128, 0:64])
                    nc.scalar.copy(v_aug[:, kb * 65:kb * 65 + 64],
                                   tp_ps[:, 0:64])
                if tb % 2 == 1:
                    si = tb // 2
                    with tc.tile_critical():
                        qoff = dyn_load(qoffs[0:1, si:si + 1], 0, T - QB)
                        nc.vector.tensor_copy(
                            qt_sel[:, si * QB:(si + 1) * QB],
                            qt_all[:, bass.ds(qoff, QB)])
                    emit_attention(si)

    _legalize_matmul_waits(nc)
    return nc


def _legalize_matmul_waits(nc):
    """walrus' LW template encodes at most one sync-wait; hoist extra waits
    from Matmult instructions onto a preceding PE NoOp (same queue, so
    ordering semantics are identical)."""
    import concourse.mybir as mybir

    for f in nc.m.functions:
        for bb in f.blocks:
            new_insts = []
            for inst in bb.instructions:
                si = inst.sync_info
                if (si is not None and si.on_wait and len(si.on_wait) >= 2):
                    for w in si.on_wait:
                        nop = mybir.InstNoOp(
                            name=nc.get_next_instruction_name(),
                            text_hint="wait_hoist", bass_nofuse=True)
                        nop.engine = inst.engine
                        nop.sync_info = mybir.SyncInfo(
                            on_wait=[w], on_update=[])
                        new_insts.append(nop)
                    inst.sync_info = mybir.SyncInfo(
                        on_wait=[], on_update=list(si.on_update or []))
                new_insts.append(inst)
            del bb.instructions[:]
            for i in new_insts:
                bb.instructions.append(i)


def _make_inputs(x, Wq, Wk, Wv):
    import ml_dtypes
    bf = ml_dtypes.bfloat16
    wkv = np.ascontiguousarray(
        np.concatenate([Wk, Wv], axis=1).astype(bf))
    wq = np.ascontiguousarray(np.asarray(Wq).astype(bf))
    ident = np.eye(128, dtype=np.float32)
    identh = np.zeros((128, 64), dtype=bf)
    identh[64:128, :] = np.eye(64, dtype=np.float32).astype(bf)
    col = np.arange(QB, dtype=np.float32)[None, :]
    row = np.arange(128, dtype=np.float32)[:, None]
    dtab = np.ascontiguousarray((col - row).astype(np.float32))  # [128, QB]

    in_maps = []
    for c in range(8):
        b, half = c // 2, c % 2
        thr, qoffs = _host_tables(half)
        thr_rep = np.ascontiguousarray(np.tile(thr[None, :], (128, 1)))
        xt = np.ascontiguousarray(np.asarray(x[b]).T.astype(bf))
        in_maps.append({
            "xt": xt, "wkv": wkv, "wq": wq, "ident": ident, "identh": identh,
            "dtab": dtab, "thr": thr_rep, "qoffs": qoffs.reshape(1, 4),
        })
    return in_maps


def kernel(x, Wq, Wk, Wv, _want_results=False, _trace=False):
    from concourse import bass_utils

    if "prog" not in _CACHE:
        _CACHE["prog"] = _build_program()
    nc = _CACHE["prog"]
    in_maps = _make_inputs(x, Wq, Wk, Wv)
    res = bass_utils.run_bass_kernel_spmd(nc, in_maps, core_ids=list(range(8)),
                                          trace=_trace)
    out = np.zeros((B, T, HD), dtype=np.float32)
    for c in range(8):
        b, half = c // 2, c % 2
        o = res.results[c]["out"]
        for si in range(4):
            qb = HALF_QBS[half][si]
            out[b, qb * QB:(qb + 1) * QB, :] = o[si * QB:(si + 1) * QB, :]
    if _want_results:
        return out, res
    return out



# revision 8
# speedup vs baseline: 1.0368x; 1.0368x over previous
"""Single-head causal attention (B=4, T=4096, n_embd=1024, head=64) on 8 trn2 cores.

Key-split scheme: core c -> batch b=c//2, half h=c%2.  Every core computes
ALL T queries of its batch but only HALF the keys: within each 512-wide
t-panel p, core h owns the contiguous 256 keys [512p+256h, 512p+256h+256),
i.e. global k-blocks {4p+2h, 4p+2h+1}.  Exact causal coverage (zero padding)
with an IDENTICAL instruction stream on every core; all per-core variation
lives in the data: a host-side roll of xt makes the owned keys sit at columns
[0:256) of every panel, and the mask table dtab[i,c] = ((c+256h) mod 512) - i
with two constant thresholds 128*(2h+{0,1}) drives the diagonal masking.

Math per panel pair (bf16 matmuls, fp32 psum):
  S^T[tk,tq] = K_blk^T.T @ Q^T          (panel-pair common k-blocks share one
                                         stationary load; psum [128,1024])
  P^T = exp(S^T / 8) -> bf16            (ScalarE, one op per k-block pair; no
                                         max-subtraction needed: S/8 ~ N(0,1))
  P^T *= (dtab >= thr)  on diagonal blocks only (VectorE)
  O_aug^T[65,1024] += V_aug_blk.T @ P^T (V_aug col 64 = ones -> row 64 of
                                         O_aug accumulates the denominator)

Perf structure (measured on HW):
 - matmuls pay ~170-280ns fixed latency + ~100ns LDWEIGHTS each -> panel
   pairs halve stationary loads; Q-projection stationary is zero-padded to
   [128,128] (64-wide stationaries are ~175ns slower per matmul).
 - software pipelining: S for k-block g+1 issues before the exp-gated AV of
   g, so the in-order PE queue never stalls on ScalarE.
 - s/proj psum tiles share one 3-deep pool ring (6 banks) + O_aug (2 banks).
 - xt is host-packed [128, pair, nt, 2*512] so each pair DMA is
   16KB-contiguous per partition, in 4 chunks so projections start early.
 - output O_aug^T [65, T] is written in bf16, split across 3 DMA queues.
Host combine (unsharding): add the two per-pair partials, divide by the
summed denominator row, un-roll core h=1's panels, transpose.
"""

import numpy as np

B, T, NE, HD = 4, 4096, 1024, 64
QB = 512            # q-panel width
KB = 128            # k-block width
NP = T // QB        # 8 panels
NPR = NP // 2       # 4 panel pairs
NT = NE // 128      # 8 contraction tiles
LT = T // 2         # local key count per core

_CACHE = {}


def _build_program():
    import concourse.bass as bass
    import concourse.mybir as mybir
    import concourse.tile as tile

    f32 = mybir.dt.float32
    bf16 = mybir.dt.bfloat16
    AF = mybir.ActivationFunctionType
    MS = bass.MemorySpace
    nc = bass.Bass("TRN2", target_bir_lowering=True, debug=False,
                   enable_asserts=False)

    # xt packed [128, pair, nt, 2*QB] (pair-major, 16KB contiguous per line)
    xt_d = nc.dram_tensor("xt", [128, NPR, NT, 2 * QB], bf16,
                          kind="ExternalInput").ap()
    wkv_d = nc.dram_tensor("wkv", [NE, 128], bf16, kind="ExternalInput").ap()
    wqp_d = nc.dram_tensor("wqp", [NE, 128], bf16, kind="ExternalInput").ap()
    identh_d = nc.dram_tensor("identh", [128, 64], bf16, kind="ExternalInput").ap()
    dtab_d = nc.dram_tensor("dtab", [128, QB], f32, kind="ExternalInput").ap()
    thr_d = nc.dram_tensor("thr", [128, 2], f32, kind="ExternalInput").ap()
    out_d = nc.dram_tensor("out", [65, T], bf16, kind="ExternalOutput").ap()

    with tile.TileContext(nc) as tc:
        with (
            tc.tile_pool(name="consts", bufs=1) as cpool,
            tc.tile_pool(name="big", bufs=1) as bigpool,
            tc.tile_pool(name="xt", bufs=2) as xtpool,
            tc.tile_pool(name="pt", bufs=3) as ptpool,
            tc.tile_pool(name="ob", bufs=2) as obpool,
            tc.tile_pool(name="sps", bufs=3, space=MS.PSUM) as spool,
            tc.tile_pool(name="ops", bufs=1, space=MS.PSUM) as opool,
        ):
            # ---- constants ----
            wkv_sb = cpool.tile([128, NT, 128], bf16)
            nc.sync.dma_start(wkv_sb[:], wkv_d.rearrange("(nt p) m -> p nt m", p=128))
            wq_sb = cpool.tile([128, NT, 128], bf16)
            nc.sync.dma_start(wq_sb[:], wqp_d.rearrange("(nt p) m -> p nt m", p=128))
            identh = cpool.tile([128, 64], bf16)
            nc.sync.dma_start(identh[:], identh_d[:])
            dtab = cpool.tile([128, QB], f32)
            nc.sync.dma_start(dtab[:], dtab_d[:])
            thr = cpool.tile([128, 2], f32)
            nc.sync.dma_start(thr[:], thr_d[:])

            # ---- persistent sbuf state ----
            kvt = bigpool.tile([128, LT], bf16)        # 0:64 K^T, 64:128 V^T
            qt = bigpool.tile([64, T], bf16)           # rolled Q^T, all panels
            v_aug = bigpool.tile([128, 16 * 65], bf16)  # V natural + ones col
            nc.vector.memset(v_aug[:], 1.0)

            for m in range(NPR):
                # ---- projections for panel pair (2m, 2m+1) ----
                xt_sbs = []
                for k in range(4):
                    xt_sb = xtpool.tile([128, 2, 2, QB], bf16, tag=f"xt{k}")
                    nc.gpsimd.dma_start(
                        xt_sb[:],
                        xt_d[:, m, 2 * k:2 * k + 2].rearrange(
                            "p nt (two t) -> p nt two t", two=2))
                    xt_sbs.append(xt_sb)
                # KV projection: own 256 cols of both panels in one matmul
                kv_ps = spool.tile([128, 2, 256], f32, tag="sps")
                for ni in range(NT):
                    nc.tensor.matmul(kv_ps[:], wkv_sb[:, ni, :],
                                     xt_sbs[ni // 2][:, ni % 2, :, 0:256],
                                     start=(ni == 0), stop=(ni == NT - 1))
                nc.vector.tensor_copy(
                    kvt[:, m * 512:(m + 1) * 512], kv_ps[:])
                # Q projection per panel (padded stationary -> full col width)
                for pan in range(2):
                    q_ps = spool.tile([128, QB], f32, tag="sps")
                    for ni in range(NT):
                        nc.tensor.matmul(q_ps[:], wq_sb[:, ni, :],
                                         xt_sbs[ni // 2][:, ni % 2, pan, :],
                                         start=(ni == 0), stop=(ni == NT - 1))
                    nc.vector.tensor_copy(
                        qt[:, (2 * m + pan) * QB:(2 * m + pan + 1) * QB],
                        q_ps[0:64, :])
                # V natural for the 4 own k-blocks of this pair
                for j in range(4):
                    lkb = 4 * m + j
                    vt_ps = spool.tile([128, 2 * QB], bf16, tag="sps")
                    nc.tensor.transpose(
                        vt_ps[:, 0:64], kvt[64:128, lkb * KB:(lkb + 1) * KB],
                        identh[64:128, 0:64])
                    nc.vector.tensor_copy(v_aug[:, lkb * 65:lkb * 65 + 64],
                                          vt_ps[:, 0:64])

                # ---- attention for panels (2m, 2m+1), software-pipelined:
                # S for group g+1 issues before the exp-gated AV of group g so
                # the in-order PE queue never stalls on ScalarE.
                o_ps = opool.tile([65, 2 * QB], f32, tag="ops")
                ncom = 4 * m + 2
                ngrp = ncom + 1   # last group = the two extra k-blocks

                def emit_s(g):
                    s_ps = spool.tile([128, 2 * QB], f32, tag="sps")
                    if g < ncom:
                        for pan in range(2):
                            nc.tensor.matmul(
                                s_ps[:, pan * QB:(pan + 1) * QB],
                                kvt[0:64, g * KB:(g + 1) * KB],
                                qt[:, (2 * m + pan) * QB:(2 * m + pan + 1) * QB],
                                start=True, stop=True)
                    else:
                        for d in range(2):
                            nc.tensor.matmul(
                                s_ps[:, d * QB:(d + 1) * QB],
                                kvt[0:64, (ncom + d) * KB:(ncom + d + 1) * KB],
                                qt[:, (2 * m + 1) * QB:(2 * m + 2) * QB],
                                start=True, stop=True)
                    return s_ps

                s_cur = emit_s(0)
                for g in range(ngrp):
                    s_next = emit_s(g + 1) if g + 1 < ngrp else None
                    pt = ptpool.tile([128, 2 * QB], bf16, tag="pt")
                    nc.scalar.activation(pt[:], s_cur[:], AF.Exp,
                                         scale=float(HD) ** -0.5)
                    if g < ncom:
                        if g >= 4 * m:
                            # diagonal k-block of panel 2m: mask its half only
                            nc.vector.scalar_tensor_tensor(
                                pt[:, 0:QB], dtab[:],
                                thr[:, g - 4 * m:g - 4 * m + 1],
                                pt[:, 0:QB],
                                mybir.AluOpType.is_ge, mybir.AluOpType.mult)
                        for pan in range(2):
                            nc.tensor.matmul(
                                o_ps[:, pan * QB:(pan + 1) * QB],
                                v_aug[:, g * 65:g * 65 + 65],
                                pt[:, pan * QB:(pan + 1) * QB],
                                start=(g == 0), stop=False,
                                skip_group_check=True)
                    else:
                        for d in range(2):
                            nc.vector.scalar_tensor_tensor(
                                pt[:, d * QB:(d + 1) * QB], dtab[:],
                                thr[:, d:d + 1],
                                pt[:, d * QB:(d + 1) * QB],
                                mybir.AluOpType.is_ge, mybir.AluOpType.mult)
                        nc.tensor.matmul(
                            o_ps[:, QB:2 * QB],
                            v_aug[:, ncom * 65:ncom * 65 + 65],
                            pt[:, 0:QB],
                            start=False, stop=False, skip_group_check=True)
                        nc.tensor.matmul(
                            o_ps[:, QB:2 * QB],
                            v_aug[:, (ncom + 1) * 65:(ncom + 1) * 65 + 65],
                            pt[:, QB:2 * QB],
                            start=False, stop=True, skip_group_check=True)
                    s_cur = s_next
                ob = obpool.tile([65, 2 * QB], bf16, tag="ob")
                nc.vector.tensor_copy(ob[:], o_ps[:])
                cols = slice(2 * m * QB, (2 * m + 2) * QB)
                nc.sync.dma_start(out_d[0:22, cols], ob[0:22, :])
                nc.scalar.dma_start(out_d[22:44, cols], ob[22:44, :])
                nc.gpsimd.dma_start(out_d[44:65, cols], ob[44:65, :])

    _legalize_matmul_waits(nc)
    return nc


def _legalize_matmul_waits(nc):
    """walrus' LW template encodes at most one sync-wait; hoist extra waits
    from Matmult instructions onto a preceding PE NoOp (same queue, so
    ordering semantics are identical)."""
    import concourse.mybir as mybir

    for f in nc.m.functions:
        for bb in f.blocks:
            new_insts = []
            for inst in bb.instructions:
                si = inst.sync_info
                if (si is not None and si.on_wait and len(si.on_wait) >= 2):
                    for w in si.on_wait:
                        nop = mybir.InstNoOp(
                            name=nc.get_next_instruction_name(),
                            text_hint="wait_hoist", bass_nofuse=True)
                        nop.engine = inst.engine
                        nop.sync_info = mybir.SyncInfo(
                            on_wait=[w], on_update=[])
                        new_insts.append(nop)
                    inst.sync_info = mybir.SyncInfo(
                        on_wait=[], on_update=list(si.on_update or []))
                new_insts.append(inst)
            del bb.instructions[:]
            for i in new_insts:
                bb.instructions.append(i)


def _make_inputs(x, Wq, Wk, Wv):
    import ml_dtypes
    bf = ml_dtypes.bfloat16
    wkv = np.ascontiguousarray(np.concatenate([Wk, Wv], axis=1).astype(bf))
    wqp = np.zeros((NE, 128), dtype=bf)
    wqp[:, 0:HD] = np.asarray(Wq).astype(bf)
    identh = np.zeros((128, 64), dtype=bf)
    identh[64:128, :] = np.eye(64, dtype=np.float32).astype(bf)

    col = np.arange(QB, dtype=np.int64)[None, :]
    row = np.arange(128, dtype=np.int64)[:, None]

    in_maps = []
    for c in range(8):
        b, h = c // 2, c % 2
        qp = (col + 256 * h) % 512               # rolled q index within panel
        dtab = np.ascontiguousarray((qp - row).astype(np.float32))
        thr = np.zeros((128, 2), dtype=np.float32)
        thr[:, 0] = 128.0 * (2 * h)
        thr[:, 1] = 128.0 * (2 * h + 1)
        xt = np.asarray(x[b]).T.astype(bf)       # [NE, T]
        xtr = xt.reshape(NE, NP, QB)
        xtr = np.roll(xtr, -256 * h, axis=2)     # per-core panel roll
        # pack [NE, NP, QB] -> [128 part, pair, nt, 2*QB]
        xp = xtr.reshape(NT, 128, NPR, 2 * QB).transpose(1, 2, 0, 3)
        in_maps.append({
            "xt": np.ascontiguousarray(xp),
            "wkv": wkv, "wqp": wqp, "identh": identh,
            "dtab": dtab, "thr": thr,
        })
    return in_maps


def kernel(x, Wq, Wk, Wv, _want_results=False, _trace=False):
    from concourse import bass_utils

    if "prog" not in _CACHE:
        _CACHE["prog"] = _build_program()
    nc = _CACHE["prog"]
    in_maps = _make_inputs(x, Wq, Wk, Wv)
    res = bass_utils.run_bass_kernel_spmd(nc, in_maps, core_ids=list(range(8)),
                                          trace=_trace)
    out = np.zeros((B, T, HD), dtype=np.float32)
    for b in range(B):
        o0 = np.asarray(res.results[2 * b]["out"], dtype=np.float32)
        o1 = np.asarray(res.results[2 * b + 1]["out"], dtype=np.float32)
        o1 = np.roll(o1.reshape(65, NP, QB), -256, axis=2).reshape(65, T)
        numer = o0[:HD] + o1[:HD]
        denom = o0[HD] + o1[HD]
        out[b] = (numer / denom).T
    if _want_results:
        return out, res
    return out


# revision 9
# speedup vs baseline: 1.1453x; 1.1047x over previous
"""Single-head causal attention (B=4, T=4096, n_embd=1024, head=64) on 8 trn2 cores.

Key-split scheme: core c -> batch b=c//2, half h=c%2.  Every core computes
ALL T queries of its batch but only HALF the keys: within each 512-wide
t-panel p, core h owns the contiguous 256 keys [512p+256h, 512p+256h+256),
i.e. global k-blocks {4p+2h, 4p+2h+1}.  Exact causal coverage (zero padding)
with an IDENTICAL instruction stream on every core; all per-core variation
lives in the data: a host-side roll of xt makes the owned keys sit at columns
[0:256) of every panel, and the mask table dtab[i,c] = ((c+256h) mod 512) - i
with two constant thresholds 128*(2h+{0,1}) drives the diagonal masking.

Math per panel pair (bf16 matmuls, fp32 psum):
  S^T[tk,tq] = K_blk^T.T @ Q^T          (panel-pair common k-blocks share one
                                         stationary load; psum [128,1024])
  P^T = exp(S^T / 8) -> bf16            (ScalarE, one op per k-block pair; no
                                         max-subtraction needed: S/8 ~ N(0,1))
  P^T *= (dtab >= thr)  on diagonal blocks only (VectorE)
  O_aug^T[65,1024] += V_aug_blk.T @ P^T (V_aug col 64 = ones -> row 64 of
                                         O_aug accumulates the denominator)

Perf structure (measured on HW):
 - matmuls pay ~170-280ns fixed latency + ~100ns LDWEIGHTS each -> panel
   pairs halve stationary loads; Q-projection stationary is zero-padded to
   [128,128] (64-wide stationaries are ~175ns slower per matmul).
 - software pipelining: S for k-block g+1 issues before the exp-gated AV of
   g, so the in-order PE queue never stalls on ScalarE.
 - s/proj psum tiles share one 3-deep pool ring (6 banks) + O_aug (2 banks);
   xt prefetch 3-deep, pt 4-deep to absorb engine jitter.
 - xt is host-packed [128, pair, nt, 2*512] so each pair DMA is
   16KB-contiguous per partition, in 4 chunks so projections start early.
 - output O_aug^T [65, T] is written in bf16, split across 3 DMA queues.
Host combine (unsharding): add the two per-pair partials, divide by the
summed denominator row, un-roll core h=1's panels, transpose.
"""

import numpy as np

B, T, NE, HD = 4, 4096, 1024, 64
QB = 512            # q-panel width
KB = 128            # k-block width
NP = T // QB        # 8 panels
NPR = NP // 2       # 4 panel pairs
NT = NE // 128      # 8 contraction tiles
LT = T // 2         # local key count per core

_CACHE = {}


def _build_program():
    import concourse.bass as bass
    import concourse.mybir as mybir
    import concourse.tile as tile

    f32 = mybir.dt.float32
    bf16 = mybir.dt.bfloat16
    AF = mybir.ActivationFunctionType
    MS = bass.MemorySpace
    nc = bass.Bass("TRN2", target_bir_lowering=True, debug=False,
                   enable_asserts=False)

    # xt packed [128, pair, nt, 2*QB] (pair-major, 16KB contiguous per line)
    xt_d = nc.dram_tensor("xt", [128, NPR, NT, 2 * QB], bf16,
                          kind="ExternalInput").ap()
    wkv_d = nc.dram_tensor("wkv", [NE, 128], bf16, kind="ExternalInput").ap()
    wqp_d = nc.dram_tensor("wqp", [NE, 128], bf16, kind="ExternalInput").ap()
    identh_d = nc.dram_tensor("identh", [128, 64], bf16, kind="ExternalInput").ap()
    dtab_d = nc.dram_tensor("dtab", [128, QB], f32, kind="ExternalInput").ap()
    thr_d = nc.dram_tensor("thr", [128, 2], f32, kind="ExternalInput").ap()
    out_d = nc.dram_tensor("out", [65, T], bf16, kind="ExternalOutput").ap()

    with tile.TileContext(nc) as tc:
        with (
            tc.tile_pool(name="consts", bufs=1) as cpool,
            tc.tile_pool(name="big", bufs=1) as bigpool,
            tc.tile_pool(name="xt", bufs=3) as xtpool,
            tc.tile_pool(name="pt", bufs=4) as ptpool,
            tc.tile_pool(name="ob", bufs=2) as obpool,
            tc.tile_pool(name="sps", bufs=3, space=MS.PSUM) as spool,
            tc.tile_pool(name="ops", bufs=1, space=MS.PSUM) as opool,
        ):
            # ---- constants ----
            wkv_sb = cpool.tile([128, NT, 128], bf16)
            nc.sync.dma_start(wkv_sb[:], wkv_d.rearrange("(nt p) m -> p nt m", p=128))
            wq_sb = cpool.tile([128, NT, 128], bf16)
            nc.sync.dma_start(wq_sb[:], wqp_d.rearrange("(nt p) m -> p nt m", p=128))
            identh = cpool.tile([128, 64], bf16)
            nc.sync.dma_start(identh[:], identh_d[:])
            dtab = cpool.tile([128, QB], f32)
            nc.sync.dma_start(dtab[:], dtab_d[:])
            thr = cpool.tile([128, 2], f32)
            nc.sync.dma_start(thr[:], thr_d[:])

            # ---- persistent sbuf state ----
            kvt = bigpool.tile([128, LT], bf16)        # 0:64 K^T, 64:128 V^T
            qt = bigpool.tile([64, T], bf16)           # rolled Q^T, all panels
            v_aug = bigpool.tile([128, 16 * 65], bf16)  # V natural + ones col
            nc.vector.memset(v_aug[:], 1.0)

            for m in range(NPR):
                # ---- projections for panel pair (2m, 2m+1) ----
                xt_sbs = []
                for k in range(4):
                    xt_sb = xtpool.tile([128, 2, 2, QB], bf16, tag=f"xt{k}")
                    nc.gpsimd.dma_start(
                        xt_sb[:],
                        xt_d[:, m, 2 * k:2 * k + 2].rearrange(
                            "p nt (two t) -> p nt two t", two=2))
                    xt_sbs.append(xt_sb)
                # KV projection: own 256 cols of both panels in one matmul
                kv_ps = spool.tile([128, 2, 256], f32, tag="sps")
                for ni in range(NT):
                    nc.tensor.matmul(kv_ps[:], wkv_sb[:, ni, :],
                                     xt_sbs[ni // 2][:, ni % 2, :, 0:256],
                                     start=(ni == 0), stop=(ni == NT - 1))
                nc.vector.tensor_copy(
                    kvt[:, m * 512:(m + 1) * 512], kv_ps[:])
                # Q projection per panel (padded stationary -> full col width)
                for pan in range(2):
                    q_ps = spool.tile([128, QB], f32, tag="sps")
                    for ni in range(NT):
                        nc.tensor.matmul(q_ps[:], wq_sb[:, ni, :],
                                         xt_sbs[ni // 2][:, ni % 2, pan, :],
                                         start=(ni == 0), stop=(ni == NT - 1))
                    nc.vector.tensor_copy(
                        qt[:, (2 * m + pan) * QB:(2 * m + pan + 1) * QB],
                        q_ps[0:64, :])
                # V natural for the 4 own k-blocks of this pair
                for j in range(4):
                    lkb = 4 * m + j
                    vt_ps = spool.tile([128, 2 * QB], bf16, tag="sps")
                    nc.tensor.transpose(
                        vt_ps[:, 0:64], kvt[64:128, lkb * KB:(lkb + 1) * KB],
                        identh[64:128, 0:64])
                    nc.vector.tensor_copy(v_aug[:, lkb * 65:lkb * 65 + 64],
                                          vt_ps[:, 0:64])

                # ---- attention for panels (2m, 2m+1), software-pipelined:
                # S for group g+1 issues before the exp-gated AV of group g so
                # the in-order PE queue never stalls on ScalarE.
                o_ps = opool.tile([65, 2 * QB], f32, tag="ops")
                ncom = 4 * m + 2
                ngrp = ncom + 1   # last group = the two extra k-blocks

                def emit_s(g):
                    s_ps = spool.tile([128, 2 * QB], f32, tag="sps")
                    if g < ncom:
                        for pan in range(2):
                            nc.tensor.matmul(
                                s_ps[:, pan * QB:(pan + 1) * QB],
                                kvt[0:64, g * KB:(g + 1) * KB],
                                qt[:, (2 * m + pan) * QB:(2 * m + pan + 1) * QB],
                                start=True, stop=True)
                    else:
                        for d in range(2):
                            nc.tensor.matmul(
                                s_ps[:, d * QB:(d + 1) * QB],
                                kvt[0:64, (ncom + d) * KB:(ncom + d + 1) * KB],
                                qt[:, (2 * m + 1) * QB:(2 * m + 2) * QB],
                                start=True, stop=True)
                    return s_ps

                s_cur = emit_s(0)
                for g in range(ngrp):
                    s_next = emit_s(g + 1) if g + 1 < ngrp else None
                    pt = ptpool.tile([128, 2 * QB], bf16, tag="pt")
                    nc.scalar.activation(pt[:], s_cur[:], AF.Exp,
                                         scale=float(HD) ** -0.5)
                    if g < ncom:
                        if g >= 4 * m:
                            # diagonal k-block of panel 2m: mask its half only
                            nc.vector.scalar_tensor_tensor(
                                pt[:, 0:QB], dtab[:],
                                thr[:, g - 4 * m:g - 4 * m + 1],
                                pt[:, 0:QB],
                                mybir.AluOpType.is_ge, mybir.AluOpType.mult)
                        for pan in range(2):
                            nc.tensor.matmul(
                                o_ps[:, pan * QB:(pan + 1) * QB],
                                v_aug[:, g * 65:g * 65 + 65],
                                pt[:, pan * QB:(pan + 1) * QB],
                                start=(g == 0), stop=False,
                                skip_group_check=True)
                    else:
                        for d in range(2):
                            nc.vector.scalar_tensor_tensor(
                                pt[:, d * QB:(d + 1) * QB], dtab[:],
                                thr[:, d:d + 1],
                                pt[:, d * QB:(d + 1) * QB],
                                mybir.AluOpType.is_ge, mybir.AluOpType.mult)
                        nc.tensor.matmul(
                            o_ps[:, QB:2 * QB],
                            v_aug[:, ncom * 65:ncom * 65 + 65],
                            pt[:, 0:QB],
                            start=False, stop=False, skip_group_check=True)
                        nc.tensor.matmul(
                            o_ps[:, QB:2 * QB],
                            v_aug[:, (ncom + 1) * 65:(ncom + 1) * 65 + 65],
                            pt[:, QB:2 * QB],
                            start=False, stop=True, skip_group_check=True)
                    s_cur = s_next
                ob = obpool.tile([65, 2 * QB], bf16, tag="ob")
                nc.vector.tensor_copy(ob[:], o_ps[:])
                cols = slice(2 * m * QB, (2 * m + 2) * QB)
                nc.sync.dma_start(out_d[0:22, cols], ob[0:22, :])
                nc.scalar.dma_start(out_d[22:44, cols], ob[22:44, :])
                nc.gpsimd.dma_start(out_d[44:65, cols], ob[44:65, :])

    _legalize_matmul_waits(nc)
    return nc


def _legalize_matmul_waits(nc):
    """walrus' LW template encodes at most one sync-wait; hoist extra waits
    from Matmult instructions onto a preceding PE NoOp (same queue, so
    ordering semantics are identical)."""
    import concourse.mybir as mybir

    for f in nc.m.functions:
        for bb in f.blocks:
            new_insts = []
            for inst in bb.instructions:
                si = inst.sync_info
                if (si is not None and si.on_wait and len(si.on_wait) >= 2):
                    for w in si.on_wait:
                        nop = mybir.InstNoOp(
                            name=nc.get_next_instruction_name(),
                            text_hint="wait_hoist", bass_nofuse=True)
                        nop.engine = inst.engine
                        nop.sync_info = mybir.SyncInfo(
                            on_wait=[w], on_update=[])
                        new_insts.append(nop)
                    inst.sync_info = mybir.SyncInfo(
                        on_wait=[], on_update=list(si.on_update or []))
                new_insts.append(inst)
            del bb.instructions[:]
            for i in new_insts:
                bb.instructions.append(i)


def _make_inputs(x, Wq, Wk, Wv):
    import ml_dtypes
    bf = ml_dtypes.bfloat16
    wkv = np.ascontiguousarray(np.concatenate([Wk, Wv], axis=1).astype(bf))
    wqp = np.zeros((NE, 128), dtype=bf)
    wqp[:, 0:HD] = np.asarray(Wq).astype(bf)
    identh = np.zeros((128, 64), dtype=bf)
    identh[64:128, :] = np.eye(64, dtype=np.float32).astype(bf)

    col = np.arange(QB, dtype=np.int64)[None, :]
    row = np.arange(128, dtype=np.int64)[:, None]

    in_maps = []
    for c in range(8):
        b, h = c // 2, c % 2
        qp = (col + 256 * h) % 512               # rolled q index within panel
        dtab = np.ascontiguousarray((qp - row).astype(np.float32))
        thr = np.zeros((128, 2), dtype=np.float32)
        thr[:, 0] = 128.0 * (2 * h)
        thr[:, 1] = 128.0 * (2 * h + 1)
        xt = np.asarray(x[b]).T.astype(bf)       # [NE, T]
        xtr = xt.reshape(NE, NP, QB)
        xtr = np.roll(xtr, -256 * h, axis=2)     # per-core panel roll
        # pack [NE, NP, QB] -> [128 part, pair, nt, 2*QB]
        xp = xtr.reshape(NT, 128, NPR, 2 * QB).transpose(1, 2, 0, 3)
        in_maps.append({
            "xt": np.ascontiguousarray(xp),
            "wkv": wkv, "wqp": wqp, "identh": identh,
            "dtab": dtab, "thr": thr,
        })
    return in_maps


def kernel(x, Wq, Wk, Wv, _want_results=False, _trace=False):
    from concourse import bass_utils

    if "prog" not in _CACHE:
        _CACHE["prog"] = _build_program()
    nc = _CACHE["prog"]
    in_maps = _make_inputs(x, Wq, Wk, Wv)
    res = bass_utils.run_bass_kernel_spmd(nc, in_maps, core_ids=list(range(8)),
                                          trace=_trace)
    out = np.zeros((B, T, HD), dtype=np.float32)
    for b in range(B):
        o0 = np.asarray(res.results[2 * b]["out"], dtype=np.float32)
        o1 = np.asarray(res.results[2 * b + 1]["out"], dtype=np.float32)
        o1 = np.roll(o1.reshape(65, NP, QB), -256, axis=2).reshape(65, T)
        numer = o0[:HD] + o1[:HD]
        denom = o0[HD] + o1[HD]
        out[b] = (numer / denom).T
    if _want_results:
        return out, res
    return out
